# revision 1
# baseline (speedup 1.0000x reference)
"""GAT + global-attention pooling on 8 Trainium2 cores (Bass/Tile SPMD).

Self-contained: hardcodes all shapes. Strategy: replicate the node table
(h = x@W plus attention logits) on every core, shard destination-node
blocks 49/core, gather source rows per edge, select-matmul segment sums,
AllReduce the pooled partials, tiny MLP tail on every core.
"""
import os
import sys

if "/opt/trn_rl_repo" not in sys.path:
    sys.path.insert(0, "/opt/trn_rl_repo")

import numpy as np

from concourse import bass, bacc, tile, mybir
from concourse.bass_utils import run_bass_kernel_spmd
from concourse.masks import make_identity

N, E, C, H, D, G = 50000, 800000, 128, 4, 32, 128
NEG = 0.2
P = 128
NBLK = 392
NP = NBLK * P
NCORES = 8
BPC = NBLK // NCORES
PB = 3  # dst-blocks per Phase-B piece
TW = 136  # table row width: 128 h | 4 a_src | 4 a_dst
f32 = mybir.dt.float32
i32 = mybir.dt.int32
AF = mybir.ActivationFunctionType
OP = mybir.AluOpType


def _host_prep(inputs):
    x = np.asarray(inputs["x"], dtype=np.float32)
    ei = np.asarray(inputs["edge_index"]).astype(np.int64)
    batch = np.asarray(inputs["batch"]).astype(np.int64)
    W = np.asarray(inputs["W"], dtype=np.float32)
    att_src = np.asarray(inputs["att_src"], dtype=np.float32)
    att_dst = np.asarray(inputs["att_dst"], dtype=np.float32)

    loops = np.arange(N, dtype=np.int64)
    src = np.concatenate([ei[0], loops]).astype(np.int32)
    dst = np.concatenate([ei[1], loops]).astype(np.int32)
    order = np.argsort(dst, kind="stable")
    src, dst = src[order], dst[order]

    cnt = np.bincount(dst // P, minlength=NBLK)
    T = int(np.max((cnt + P - 1) // P))

    PAD_SRC = NP - 1
    idx_src = np.full((NBLK, T * P), PAD_SRC, dtype=np.int32)
    idx_dst = np.full((NBLK, T * P), PAD_SRC, dtype=np.int32)
    dstloc = np.full((NBLK, T * P), P - 1, dtype=np.int32)
    starts = np.concatenate([[0], np.cumsum(cnt)]).astype(np.int64)
    for b in range(NBLK):
        s, e = starts[b], starts[b + 1]
        idx_src[b, : e - s] = src[s:e]
        idx_dst[b, : e - s] = dst[s:e]
        dstloc[b, : e - s] = dst[s:e] - b * P

    def core_layout(a):
        # [NBLK, T*P] -> per-core [P, BPC*T]; element [p, j*T+t] = edge (blk j, chunk t, lane p)
        a = a.reshape(NBLK, T, P).transpose(0, 2, 1)  # [NBLK, P, T]
        a = a.reshape(NCORES, BPC, P, T).transpose(0, 2, 1, 3)  # [NCORES, P, BPC, T]
        return np.ascontiguousarray(a.reshape(NCORES, P, BPC * T))

    isrc_c = core_layout(idx_src)
    idst_c = core_layout(idx_dst)
    dloc_c = core_layout(dstloc).astype(np.float32)

    batchloc = np.full(NP, 255, dtype=np.int64)
    batchloc[:N] = batch
    bloc_c = np.ascontiguousarray(
        batchloc.reshape(NCORES, BPC, P).transpose(0, 2, 1)
    ).astype(np.float32)  # [NCORES, P, BPC]

    xT = np.zeros((C, NP), dtype=np.float32)
    xT[:, :N] = x.T

    Ablk = np.zeros((C, 2 * H), dtype=np.float32)
    for hh in range(H):
        Ablk[hh * D : (hh + 1) * D, hh] = att_src[hh]
        Ablk[hh * D : (hh + 1) * D, H + hh] = att_dst[hh]

    rep = {
        "xT": xT,
        "w": np.ascontiguousarray(W),
        "wT": np.ascontiguousarray(W.T),
        "ablk": Ablk,
        "biasM": np.tile(np.asarray(inputs["bias"], np.float32)[None, :], (P, 1)),
        "gwM": np.tile(np.asarray(inputs["gate_w"], np.float32)[:, 0][None, :], (P, 1)),
        "gateb": np.full((P, 1), np.asarray(inputs["gate_b"], np.float32)[0], np.float32),
        "w1": np.asarray(inputs["w1"], np.float32),
        "b1c": np.ascontiguousarray(np.asarray(inputs["b1"], np.float32)[:, None]),
        "w2": np.asarray(inputs["w2"], np.float32),
        "b2c": np.full((P, 1), np.asarray(inputs["b2"], np.float32)[0], np.float32),
    }
    per_core = [
        {"isrc": isrc_c[c], "idst": idst_c[c], "dloc": dloc_c[c], "bloc": bloc_c[c]}
        for c in range(NCORES)
    ]
    return T, rep, per_core


def _build_program(T):
    CT = BPC * T
    nc = bacc.Bacc()
    xT_d = nc.declare_dram_parameter("xT", [C, NP], f32, False)
    w_d = nc.declare_dram_parameter("w", [C, C], f32, False)
    wT_d = nc.declare_dram_parameter("wT", [C, C], f32, False)
    ablk_d = nc.declare_dram_parameter("ablk", [C, 2 * H], f32, False)
    biasM_d = nc.declare_dram_parameter("biasM", [P, C], f32, False)
    gwM_d = nc.declare_dram_parameter("gwM", [P, C], f32, False)
    gateb_d = nc.declare_dram_parameter("gateb", [P, 1], f32, False)
    w1_d = nc.declare_dram_parameter("w1", [C, 50], f32, False)
    b1c_d = nc.declare_dram_parameter("b1c", [50, 1], f32, False)
    w2_d = nc.declare_dram_parameter("w2", [50, 1], f32, False)
    b2c_d = nc.declare_dram_parameter("b2c", [P, 1], f32, False)
    isrc_d = nc.declare_dram_parameter("isrc", [P, CT], i32, False)
    idst_d = nc.declare_dram_parameter("idst", [P, CT], i32, False)
    dloc_d = nc.declare_dram_parameter("dloc", [P, CT], f32, False)
    bloc_d = nc.declare_dram_parameter("bloc", [P, BPC], f32, False)
    out_d = nc.declare_dram_parameter("out", [G, 1], f32, True)
    table = nc.dram_tensor("table", [NP, TW], f32)

    with tile.TileContext(nc) as tc:
        with tc.tile_pool(name="consts", bufs=1) as consts, \
             tc.tile_pool(name="gt", bufs=2) as gtp, \
             tc.tile_pool(name="adst", bufs=2) as adp, \
             tc.tile_pool(name="s01", bufs=2) as s01p, \
             tc.tile_pool(name="nrm", bufs=3) as nrmp, \
             tc.tile_pool(name="dram", bufs=1, space="DRAM") as dpool:

            # ---- Phase A: table[n] = [x_n @ W | a_src_n | a_dst_n] ----
            rhsBig = consts.tile([C, TW], f32)
            wT_sb = consts.tile([C, C], f32)
            ablk_sb = consts.tile([C, 2 * H], f32)
            nc.sync.dma_start(rhsBig[:, 0:128], w_d[:])
            nc.sync.dma_start(wT_sb[:], wT_d[:])
            nc.sync.dma_start(ablk_sb[:], ablk_d[:])
            with tc.tile_pool(name="psWaP", bufs=1, space="PSUM") as psWaP, \
                 tc.tile_pool(name="xb", bufs=4) as xbp, \
                 tc.tile_pool(name="tout", bufs=4) as toutp, \
                 tc.tile_pool(name="psA", bufs=4, space="PSUM") as psA:
                psWa = psWaP.tile([C, 2 * H], f32)
                nc.tensor.matmul(out=psWa[:], lhsT=wT_sb[:], rhs=ablk_sb[:],
                                 start=True, stop=True)
                nc.scalar.activation(out=rhsBig[:, 128:136], in_=psWa[:],
                                     func=AF.Copy)

                for b in range(NBLK):
                    xb = xbp.tile([C, P], f32)
                    nc.sync.dma_start(xb[:], xT_d[:, b * P : (b + 1) * P])
                    ps = psA.tile([P, TW], f32)
                    nc.tensor.matmul(out=ps[:], lhsT=xb[:], rhs=rhsBig[:],
                                     start=True, stop=True)
                    tout = toutp.tile([P, TW], f32)
                    nc.scalar.activation(out=tout[:], in_=ps[:], func=AF.Copy)
                    nc.sync.dma_start(table[b * P : (b + 1) * P, :], tout[:])

            # pad rows: a_src = -1e9 so padded edges contribute exp(..) = 0
            negt = consts.tile([P, 4], f32)
            nc.vector.memset(negt[:], -1e9)
            nc.sync.dma_start(table[N : N + P, 128:132], negt[:])
            nc.sync.dma_start(table[N + P : NP, 128:132], negt[0 : NP - N - P, :])

            # ---- Phase B setup ----
            isrc_sb = consts.tile([P, CT], i32)
            idst_sb = consts.tile([P, CT], i32)
            dloc_sb = consts.tile([P, CT], f32)
            bloc_sb = consts.tile([P, BPC], f32)
            biasM_sb = consts.tile([P, C], f32)
            gwM_sb = consts.tile([P, C], f32)
            gateb_sb = consts.tile([P, 1], f32)
            for sb, dr in [(isrc_sb, isrc_d), (idst_sb, idst_d), (dloc_sb, dloc_d),
                           (bloc_sb, bloc_d), (biasM_sb, biasM_d), (gwM_sb, gwM_d),
                           (gateb_sb, gateb_d)]:
                nc.sync.dma_start(sb[:], dr[:])
            iotaI = consts.tile([P, 1, P], i32)
            nc.gpsimd.iota(iotaI[:], pattern=[[1, P]], base=0, channel_multiplier=0)
            iotaF = consts.tile([P, 1, P], f32)
            nc.vector.tensor_copy(out=iotaF[:], in_=iotaI[:])

            x2All = consts.tile([P, BPC, 129], f32)
            gateAll = consts.tile([P, BPC], f32)

            pieces = []
            j0 = 0
            while j0 < BPC:
                nb = min(PB, BPC - j0)
                pieces.append((j0, nb))
                j0 += nb

            # ---- Phase B: per dst-block gather + weighted segment sums ----
            psB_cm = tc.tile_pool(name="psB", bufs=2, space="PSUM")
            psB = psB_cm.__enter__()
            for (j0, nb) in pieces:
                cols = nb * T
                c0 = j0 * T
                Gt = gtp.tile([P, cols, TW], f32)
                Adst = adp.tile([P, cols, 4], f32)
                for cc in range(cols):
                    nc.gpsimd.indirect_dma_start(
                        out=Gt[:, cc, :], out_offset=None, in_=table[:, :],
                        in_offset=bass.IndirectOffsetOnAxis(
                            ap=isrc_sb[:, c0 + cc : c0 + cc + 1], axis=0),
                        element_offset=0)
                    nc.gpsimd.indirect_dma_start(
                        out=Adst[:, cc, :], out_offset=None, in_=table[:, :],
                        in_offset=bass.IndirectOffsetOnAxis(
                            ap=idst_sb[:, c0 + cc : c0 + cc + 1], axis=0),
                        element_offset=132)

                w4 = Gt[:, :, 128:132]
                nc.vector.tensor_tensor(out=w4, in0=w4, in1=Adst[:], op=OP.add)
                nc.vector.scalar_tensor_tensor(out=w4, in0=w4, scalar=NEG, in1=w4,
                                               op0=OP.mult, op1=OP.max)
                nc.scalar.activation(out=w4, in_=w4, func=AF.Exp)
                gt4 = Gt[:, :, 0:128].rearrange("p a (h d) -> p a h d", d=D)
                nc.vector.tensor_tensor(out=gt4, in0=gt4,
                                        in1=w4.to_broadcast([P, cols, H, D]),
                                        op=OP.mult)

                S01 = s01p.tile([P, cols, P], f32)
                nc.vector.tensor_tensor(
                    out=S01[:],
                    in0=dloc_sb[:, c0 : c0 + cols].to_broadcast([P, cols, P]),
                    in1=iotaF[:].to_broadcast([P, cols, P]),
                    op=OP.is_equal)

                for jj in range(nb):
                    j = j0 + jj
                    psb = psB.tile([P, 132], f32)
                    for t in range(T):
                        cc = jj * T + t
                        nc.tensor.matmul(out=psb[:], lhsT=S01[:, cc, :],
                                         rhs=Gt[:, cc, 0:132],
                                         start=(t == 0), stop=(t == T - 1))
                    den = nrmp.tile([P, 4], f32)
                    nc.scalar.activation(out=den[:], in_=psb[:, 128:132],
                                         func=AF.Copy, bias=1e-16)
                    rden = nrmp.tile([P, 4], f32)
                    nc.vector.reciprocal(out=rden[:], in_=den[:])
                    xslot = x2All[:, j, 0:128]
                    nc.vector.tensor_tensor(
                        out=xslot.rearrange("p (h d) -> p h d", d=D),
                        in0=psb[:, 0:128].rearrange("p (h d) -> p h d", d=D),
                        in1=rden[:].to_broadcast([P, H, D]), op=OP.mult)
                    nc.vector.tensor_tensor(out=xslot, in0=xslot, in1=biasM_sb[:],
                                            op=OP.add)
                    # elu(x) = max(exp(min(x,0)) - 1, x); min(x,0) = -relu(-x)
                    tmp = nrmp.tile([P, C], f32)
                    nc.scalar.activation(out=tmp[:], in_=xslot, func=AF.Relu,
                                         scale=-1.0)
                    nc.scalar.activation(out=tmp[:], in_=tmp[:], func=AF.Exp,
                                         scale=-1.0)
                    nc.vector.scalar_tensor_tensor(out=xslot, in0=tmp[:], scalar=-1.0,
                                                   in1=xslot, op0=OP.add, op1=OP.max)
                    gsc = nrmp.tile([P, C], f32)
                    nc.vector.tensor_tensor(out=gsc[:], in0=xslot, in1=gwM_sb[:],
                                            op=OP.mult)
                    nc.vector.tensor_reduce(out=gateAll[:, j : j + 1], in_=gsc[:],
                                            axis=mybir.AxisListType.X, op=OP.add)

            psB_cm.__exit__(None, None, None)

            # ---- Phase C: gated pooling + AllReduce + MLP ----
            psC_cm = tc.tile_pool(name="psC", bufs=1, space="PSUM")
            psC = psC_cm.__enter__()
            nc.vector.tensor_tensor(out=gateAll[:], in0=gateAll[:],
                                    in1=gateb_sb[:].to_broadcast([P, BPC]),
                                    op=OP.add)
            nc.scalar.activation(out=gateAll[:], in_=gateAll[:], func=AF.Exp)
            x2v = x2All[:, :, 0:128]
            nc.vector.tensor_tensor(out=x2v, in0=x2v,
                                    in1=gateAll[:].to_broadcast([P, BPC, 128]),
                                    op=OP.mult)
            nc.vector.tensor_copy(out=x2All[:, :, 128], in_=gateAll[:])

            S01g = consts.tile([P, BPC, P], f32)
            nc.vector.tensor_tensor(
                out=S01g[:], in0=bloc_sb[:].to_broadcast([P, BPC, P]),
                in1=iotaF[:].to_broadcast([P, BPC, P]), op=OP.is_equal)

            psPool = psC.tile([P, 129], f32)
            for j in range(BPC):
                nc.tensor.matmul(out=psPool[:], lhsT=S01g[:, j, :],
                                 rhs=x2All[:, j, :],
                                 start=(j == 0), stop=(j == BPC - 1))
            poolS = consts.tile([P, 129], f32)
            nc.scalar.activation(out=poolS[:], in_=psPool[:], func=AF.Copy)

            cc_in = dpool.tile([P, 129], f32)
            cc_out = dpool.tile([P, 129], f32)
            nc.gpsimd.dma_start(cc_in[:], poolS[:])
            nc.gpsimd.collective_compute(
                "AllReduce", OP.add, replica_groups=[list(range(NCORES))],
                ins=[cc_in.opt()], outs=[cc_out.opt()])
            poolR = consts.tile([P, 129], f32)
            nc.gpsimd.dma_start(poolR[:], cc_out[:])

            den1 = consts.tile([P, 1], f32)
            nc.scalar.activation(out=den1[:], in_=poolR[:, 128:129], func=AF.Copy,
                                 bias=1e-16)
            rdg = consts.tile([P, 1], f32)
            nc.vector.reciprocal(out=rdg[:], in_=den1[:])
            pooledN = consts.tile([P, C], f32)
            nc.scalar.activation(out=pooledN[:], in_=poolR[:, 0:128], func=AF.Copy,
                                 scale=rdg[:])

            ident = consts.tile([P, P], f32)
            make_identity(nc, ident[:])
            psTr = psC.tile([P, P], f32)
            nc.tensor.transpose(out=psTr[:], in_=pooledN[:], identity=ident[:])
            pooledT = consts.tile([P, P], f32)
            nc.scalar.activation(out=pooledT[:], in_=psTr[:], func=AF.Copy)

            w1_sb = consts.tile([C, 50], f32)
            b1c_sb = consts.tile([50, 1], f32)
            w2_sb = consts.tile([50, 1], f32)
            b2c_sb = consts.tile([P, 1], f32)
            for sb, dr in [(w1_sb, w1_d), (b1c_sb, b1c_d), (w2_sb, w2_d),
                           (b2c_sb, b2c_d)]:
                nc.sync.dma_start(sb[:], dr[:])
            psH = psC.tile([50, P], f32)
            nc.tensor.matmul(out=psH[:], lhsT=w1_sb[:], rhs=pooledT[:],
                             start=True, stop=True)
            h1s = consts.tile([50, P], f32)
            nc.scalar.activation(out=h1s[:], in_=psH[:], func=AF.Relu,
                                 bias=b1c_sb[:])
            psO = psC.tile([P, 1], f32)
            nc.tensor.matmul(out=psO[:], lhsT=h1s[:], rhs=w2_sb[:],
                             start=True, stop=True)
            outS = consts.tile([P, 1], f32)
            nc.scalar.activation(out=outS[:], in_=psO[:], func=AF.Identity,
                                 bias=b2c_sb[:])
            nc.sync.dma_start(out_d[:], outS[:])
            psC_cm.__exit__(None, None, None)
    return nc


LAST_EXEC_NS = None


def kernel(**inputs):
    global LAST_EXEC_NS
    T, rep, per_core = _host_prep(inputs)
    nc = _build_program(T)
    in_maps = [dict(rep, **per_core[c]) for c in range(NCORES)]
    nc.finalize()
    trace = os.environ.get("BASS_TRACE") == "1"
    res = run_bass_kernel_spmd(nc, in_maps, list(range(NCORES)), trace=trace)
    LAST_EXEC_NS = getattr(res, "exec_time_ns", None)
    return np.asarray(res.results[0]["out"], dtype=np.float32)



# revision 2
# speedup vs baseline: 4.3558x; 4.3558x over previous
"""GAT + global-attention pooling on 8 Trainium2 cores (Bass/Tile SPMD).

Self-contained: hardcodes all shapes. Strategy: replicate the node table
(h = x@W plus attention logits) on every core, shard destination-node
blocks 49/core, gather source rows per edge, select-matmul segment sums,
AllReduce the pooled partials, tiny MLP tail on every core.
"""
import os
import sys

if "/opt/trn_rl_repo" not in sys.path:
    sys.path.insert(0, "/opt/trn_rl_repo")

import numpy as np

from concourse import bass, bacc, tile, mybir
from concourse.bass_utils import run_bass_kernel_spmd
from concourse.masks import make_identity

N, E, C, H, D, G = 50000, 800000, 128, 4, 32, 128
NEG = 0.2
P = 128
NBLK = 392
NP = NBLK * P
NCORES = 8
BPC = NBLK // NCORES
PB = 3  # dst-blocks per Phase-B piece
TW = 136  # table row width: 128 h | 4 a_src | 4 a_dst
f32 = mybir.dt.float32
i32 = mybir.dt.int32
AF = mybir.ActivationFunctionType
OP = mybir.AluOpType


def _host_prep(inputs):
    x = np.asarray(inputs["x"], dtype=np.float32)
    ei = np.asarray(inputs["edge_index"]).astype(np.int64)
    batch = np.asarray(inputs["batch"]).astype(np.int64)
    W = np.asarray(inputs["W"], dtype=np.float32)
    att_src = np.asarray(inputs["att_src"], dtype=np.float32)
    att_dst = np.asarray(inputs["att_dst"], dtype=np.float32)

    loops = np.arange(N, dtype=np.int64)
    src = np.concatenate([ei[0], loops]).astype(np.int32)
    dst = np.concatenate([ei[1], loops]).astype(np.int32)
    order = np.argsort(dst, kind="stable")
    src, dst = src[order], dst[order]

    cnt = np.bincount(dst // P, minlength=NBLK)
    T = int(np.max((cnt + P - 1) // P))

    PAD_SRC = NP - 1
    idx_src = np.full((NBLK, T * P), PAD_SRC, dtype=np.int32)
    idx_dst = np.full((NBLK, T * P), PAD_SRC, dtype=np.int32)
    dstloc = np.full((NBLK, T * P), P - 1, dtype=np.int32)
    starts = np.concatenate([[0], np.cumsum(cnt)]).astype(np.int64)
    for b in range(NBLK):
        s, e = starts[b], starts[b + 1]
        idx_src[b, : e - s] = src[s:e]
        idx_dst[b, : e - s] = dst[s:e]
        dstloc[b, : e - s] = dst[s:e] - b * P

    def core_layout(a):
        # [NBLK, T*P] -> per-core [P, BPC*T]; element [p, j*T+t] = edge (blk j, chunk t, lane p)
        a = a.reshape(NBLK, T, P).transpose(0, 2, 1)  # [NBLK, P, T]
        a = a.reshape(NCORES, BPC, P, T).transpose(0, 2, 1, 3)  # [NCORES, P, BPC, T]
        return np.ascontiguousarray(a.reshape(NCORES, P, BPC * T))

    isrc_c = core_layout(idx_src)
    idst_c = core_layout(idx_dst)
    dloc_c = core_layout(dstloc).astype(np.float32)

    batchloc = np.full(NP, 255, dtype=np.int64)
    batchloc[:N] = batch
    bloc_c = np.ascontiguousarray(
        batchloc.reshape(NCORES, BPC, P).transpose(0, 2, 1)
    ).astype(np.float32)  # [NCORES, P, BPC]

    xT = np.zeros((C, NP), dtype=np.float32)
    xT[:, :N] = x.T

    Ablk = np.zeros((C, 2 * H), dtype=np.float32)
    for hh in range(H):
        Ablk[hh * D : (hh + 1) * D, hh] = att_src[hh]
        Ablk[hh * D : (hh + 1) * D, H + hh] = att_dst[hh]

    rep = {
        "xT": xT,
        "w": np.ascontiguousarray(W),
        "wT": np.ascontiguousarray(W.T),
        "ablk": Ablk,
        "biasM": np.tile(np.asarray(inputs["bias"], np.float32)[None, :], (P, 1)),
        "gwM": np.tile(np.asarray(inputs["gate_w"], np.float32)[:, 0][None, :], (P, 1)),
        "gateb": np.full((P, 1), np.asarray(inputs["gate_b"], np.float32)[0], np.float32),
        "w1": np.asarray(inputs["w1"], np.float32),
        "b1c": np.ascontiguousarray(np.asarray(inputs["b1"], np.float32)[:, None]),
        "w2": np.asarray(inputs["w2"], np.float32),
        "b2c": np.full((P, 1), np.asarray(inputs["b2"], np.float32)[0], np.float32),
    }
    per_core = [
        {"isrc": isrc_c[c], "idst": idst_c[c], "dloc": dloc_c[c], "bloc": bloc_c[c]}
        for c in range(NCORES)
    ]
    return T, rep, per_core


def _build_program(T):
    CT = BPC * T
    nc = bacc.Bacc()
    xT_d = nc.declare_dram_parameter("xT", [C, NP], f32, False)
    w_d = nc.declare_dram_parameter("w", [C, C], f32, False)
    wT_d = nc.declare_dram_parameter("wT", [C, C], f32, False)
    ablk_d = nc.declare_dram_parameter("ablk", [C, 2 * H], f32, False)
    biasM_d = nc.declare_dram_parameter("biasM", [P, C], f32, False)
    gwM_d = nc.declare_dram_parameter("gwM", [P, C], f32, False)
    gateb_d = nc.declare_dram_parameter("gateb", [P, 1], f32, False)
    w1_d = nc.declare_dram_parameter("w1", [C, 50], f32, False)
    b1c_d = nc.declare_dram_parameter("b1c", [50, 1], f32, False)
    w2_d = nc.declare_dram_parameter("w2", [50, 1], f32, False)
    b2c_d = nc.declare_dram_parameter("b2c", [P, 1], f32, False)
    isrc_d = nc.declare_dram_parameter("isrc", [P, CT], i32, False)
    idst_d = nc.declare_dram_parameter("idst", [P, CT], i32, False)
    dloc_d = nc.declare_dram_parameter("dloc", [P, CT], f32, False)
    bloc_d = nc.declare_dram_parameter("bloc", [P, BPC], f32, False)
    out_d = nc.declare_dram_parameter("out", [G, 1], f32, True)
    table = nc.dram_tensor("table", [NP, TW], f32)

    with tile.TileContext(nc) as tc:
        with tc.tile_pool(name="consts", bufs=1) as consts, \
             tc.tile_pool(name="gt", bufs=2) as gtp, \
             tc.tile_pool(name="adst", bufs=2) as adp, \
             tc.tile_pool(name="s01", bufs=2) as s01p, \
             tc.tile_pool(name="nrm", bufs=3) as nrmp, \
             tc.tile_pool(name="dram", bufs=1, space="DRAM") as dpool:

            # ---- Phase A: table[n] = [x_n @ W | a_src_n | a_dst_n] ----
            rhsBig = consts.tile([C, TW], f32)
            wT_sb = consts.tile([C, C], f32)
            ablk_sb = consts.tile([C, 2 * H], f32)
            nc.sync.dma_start(rhsBig[:, 0:128], w_d[:])
            nc.sync.dma_start(wT_sb[:], wT_d[:])
            nc.sync.dma_start(ablk_sb[:], ablk_d[:])
            with tc.tile_pool(name="psWaP", bufs=1, space="PSUM") as psWaP, \
                 tc.tile_pool(name="xb", bufs=4) as xbp, \
                 tc.tile_pool(name="tout", bufs=4) as toutp, \
                 tc.tile_pool(name="psA", bufs=4, space="PSUM") as psA:
                psWa = psWaP.tile([C, 2 * H], f32)
                nc.tensor.matmul(out=psWa[:], lhsT=wT_sb[:], rhs=ablk_sb[:],
                                 start=True, stop=True)
                nc.scalar.activation(out=rhsBig[:, 128:136], in_=psWa[:],
                                     func=AF.Copy)

                for b in range(NBLK):
                    xb = xbp.tile([C, P], f32)
                    nc.sync.dma_start(xb[:], xT_d[:, b * P : (b + 1) * P])
                    ps = psA.tile([P, TW], f32)
                    nc.tensor.matmul(out=ps[:], lhsT=xb[:], rhs=rhsBig[:],
                                     start=True, stop=True)
                    tout = toutp.tile([P, TW], f32)
                    nc.scalar.activation(out=tout[:], in_=ps[:], func=AF.Copy)
                    nc.sync.dma_start(table[b * P : (b + 1) * P, :], tout[:])

            # pad rows: a_src = -1e9 so padded edges contribute exp(..) = 0
            negt = consts.tile([P, 4], f32)
            nc.vector.memset(negt[:], -1e9)
            nc.sync.dma_start(table[N : N + P, 128:132], negt[:])
            nc.sync.dma_start(table[N + P : NP, 128:132], negt[0 : NP - N - P, :])

            # ---- Phase B setup ----
            isrc_sb = consts.tile([P, CT], i32)
            idst_sb = consts.tile([P, CT], i32)
            dloc_sb = consts.tile([P, CT], f32)
            bloc_sb = consts.tile([P, BPC], f32)
            biasM_sb = consts.tile([P, C], f32)
            gwM_sb = consts.tile([P, C], f32)
            gateb_sb = consts.tile([P, 1], f32)
            for sb, dr in [(isrc_sb, isrc_d), (idst_sb, idst_d), (dloc_sb, dloc_d),
                           (bloc_sb, bloc_d), (biasM_sb, biasM_d), (gwM_sb, gwM_d),
                           (gateb_sb, gateb_d)]:
                nc.sync.dma_start(sb[:], dr[:])
            iotaI = consts.tile([P, 1, P], i32)
            nc.gpsimd.iota(iotaI[:], pattern=[[1, P]], base=0, channel_multiplier=0)
            iotaF = consts.tile([P, 1, P], f32)
            nc.vector.tensor_copy(out=iotaF[:], in_=iotaI[:])

            x2All = consts.tile([P, BPC, 129], f32)
            gateAll = consts.tile([P, BPC], f32)

            pieces = []
            j0 = 0
            while j0 < BPC:
                nb = min(PB, BPC - j0)
                pieces.append((j0, nb))
                j0 += nb

            # ---- Phase B: per dst-block gather + weighted segment sums ----
            psB_cm = tc.tile_pool(name="psB", bufs=2, space="PSUM")
            psB = psB_cm.__enter__()
            for (j0, nb) in pieces:
                cols = nb * T
                c0 = j0 * T
                Gt = gtp.tile([P, cols, TW], f32)
                Adst = adp.tile([P, cols, 4], f32)
                for cc in range(cols):
                    nc.gpsimd.indirect_dma_start(
                        out=Gt[:, cc, :], out_offset=None, in_=table[:, :],
                        in_offset=bass.IndirectOffsetOnAxis(
                            ap=isrc_sb[:, c0 + cc : c0 + cc + 1], axis=0),
                        element_offset=0)
                    nc.gpsimd.indirect_dma_start(
                        out=Adst[:, cc, :], out_offset=None, in_=table[:, :],
                        in_offset=bass.IndirectOffsetOnAxis(
                            ap=idst_sb[:, c0 + cc : c0 + cc + 1], axis=0),
                        element_offset=132)

                w4 = Gt[:, :, 128:132]
                nc.vector.tensor_tensor(out=w4, in0=w4, in1=Adst[:], op=OP.add)
                nc.vector.scalar_tensor_tensor(out=w4, in0=w4, scalar=NEG, in1=w4,
                                               op0=OP.mult, op1=OP.max)
                nc.scalar.activation(out=w4, in_=w4, func=AF.Exp)
                gt4 = Gt[:, :, 0:128].rearrange("p a (h d) -> p a h d", d=D)
                nc.vector.tensor_tensor(out=gt4, in0=gt4,
                                        in1=w4.to_broadcast([P, cols, H, D]),
                                        op=OP.mult)

                S01 = s01p.tile([P, cols, P], f32)
                nc.vector.tensor_tensor(
                    out=S01[:],
                    in0=dloc_sb[:, c0 : c0 + cols].to_broadcast([P, cols, P]),
                    in1=iotaF[:].to_broadcast([P, cols, P]),
                    op=OP.is_equal)

                for jj in range(nb):
                    j = j0 + jj
                    psb = psB.tile([P, 132], f32)
                    for t in range(T):
                        cc = jj * T + t
                        nc.tensor.matmul(out=psb[:], lhsT=S01[:, cc, :],
                                         rhs=Gt[:, cc, 0:132],
                                         start=(t == 0), stop=(t == T - 1))
                    den = nrmp.tile([P, 4], f32)
                    nc.scalar.activation(out=den[:], in_=psb[:, 128:132],
                                         func=AF.Copy, bias=1e-16)
                    rden = nrmp.tile([P, 4], f32)
                    nc.vector.reciprocal(out=rden[:], in_=den[:])
                    xslot = x2All[:, j, 0:128]
                    nc.vector.tensor_tensor(
                        out=xslot.rearrange("p (h d) -> p h d", d=D),
                        in0=psb[:, 0:128].rearrange("p (h d) -> p h d", d=D),
                        in1=rden[:].to_broadcast([P, H, D]), op=OP.mult)
                    nc.vector.tensor_tensor(out=xslot, in0=xslot, in1=biasM_sb[:],
                                            op=OP.add)
                    # elu(x) = max(exp(min(x,0)) - 1, x); min(x,0) = -relu(-x)
                    tmp = nrmp.tile([P, C], f32)
                    nc.scalar.activation(out=tmp[:], in_=xslot, func=AF.Relu,
                                         scale=-1.0)
                    nc.scalar.activation(out=tmp[:], in_=tmp[:], func=AF.Exp,
                                         scale=-1.0)
                    nc.vector.scalar_tensor_tensor(out=xslot, in0=tmp[:], scalar=-1.0,
                                                   in1=xslot, op0=OP.add, op1=OP.max)
                    gsc = nrmp.tile([P, C], f32)
                    nc.vector.tensor_tensor(out=gsc[:], in0=xslot, in1=gwM_sb[:],
                                            op=OP.mult)
                    nc.vector.tensor_reduce(out=gateAll[:, j : j + 1], in_=gsc[:],
                                            axis=mybir.AxisListType.X, op=OP.add)

            psB_cm.__exit__(None, None, None)

            # ---- Phase C: gated pooling + AllReduce + MLP ----
            psC_cm = tc.tile_pool(name="psC", bufs=1, space="PSUM")
            psC = psC_cm.__enter__()
            nc.vector.tensor_tensor(out=gateAll[:], in0=gateAll[:],
                                    in1=gateb_sb[:].to_broadcast([P, BPC]),
                                    op=OP.add)
            nc.scalar.activation(out=gateAll[:], in_=gateAll[:], func=AF.Exp)
            x2v = x2All[:, :, 0:128]
            nc.vector.tensor_tensor(out=x2v, in0=x2v,
                                    in1=gateAll[:].to_broadcast([P, BPC, 128]),
                                    op=OP.mult)
            nc.vector.tensor_copy(out=x2All[:, :, 128], in_=gateAll[:])

            S01g = consts.tile([P, BPC, P], f32)
            nc.vector.tensor_tensor(
                out=S01g[:], in0=bloc_sb[:].to_broadcast([P, BPC, P]),
                in1=iotaF[:].to_broadcast([P, BPC, P]), op=OP.is_equal)

            psPool = psC.tile([P, 129], f32)
            for j in range(BPC):
                nc.tensor.matmul(out=psPool[:], lhsT=S01g[:, j, :],
                                 rhs=x2All[:, j, :],
                                 start=(j == 0), stop=(j == BPC - 1))
            poolS = consts.tile([P, 129], f32)
            nc.scalar.activation(out=poolS[:], in_=psPool[:], func=AF.Copy)

            cc_in = dpool.tile([P, 129], f32)
            cc_out = dpool.tile([P, 129], f32)
            nc.gpsimd.dma_start(cc_in[:], poolS[:])
            nc.gpsimd.collective_compute(
                "AllReduce", OP.add, replica_groups=[list(range(NCORES))],
                ins=[cc_in.opt()], outs=[cc_out.opt()])
            poolR = consts.tile([P, 129], f32)
            nc.gpsimd.dma_start(poolR[:], cc_out[:])

            den1 = consts.tile([P, 1], f32)
            nc.scalar.activation(out=den1[:], in_=poolR[:, 128:129], func=AF.Copy,
                                 bias=1e-16)
            rdg = consts.tile([P, 1], f32)
            nc.vector.reciprocal(out=rdg[:], in_=den1[:])
            pooledN = consts.tile([P, C], f32)
            nc.scalar.activation(out=pooledN[:], in_=poolR[:, 0:128], func=AF.Copy,
                                 scale=rdg[:])

            ident = consts.tile([P, P], f32)
            make_identity(nc, ident[:])
            psTr = psC.tile([P, P], f32)
            nc.tensor.transpose(out=psTr[:], in_=pooledN[:], identity=ident[:])
            pooledT = consts.tile([P, P], f32)
            nc.scalar.activation(out=pooledT[:], in_=psTr[:], func=AF.Copy)

            w1_sb = consts.tile([C, 50], f32)
            b1c_sb = consts.tile([50, 1], f32)
            w2_sb = consts.tile([50, 1], f32)
            b2c_sb = consts.tile([P, 1], f32)
            for sb, dr in [(w1_sb, w1_d), (b1c_sb, b1c_d), (w2_sb, w2_d),
                           (b2c_sb, b2c_d)]:
                nc.sync.dma_start(sb[:], dr[:])
            psH = psC.tile([50, P], f32)
            nc.tensor.matmul(out=psH[:], lhsT=w1_sb[:], rhs=pooledT[:],
                             start=True, stop=True)
            h1s = consts.tile([50, P], f32)
            nc.scalar.activation(out=h1s[:], in_=psH[:], func=AF.Relu,
                                 bias=b1c_sb[:])
            psO = psC.tile([P, 1], f32)
            nc.tensor.matmul(out=psO[:], lhsT=h1s[:], rhs=w2_sb[:],
                             start=True, stop=True)
            outS = consts.tile([P, 1], f32)
            nc.scalar.activation(out=outS[:], in_=psO[:], func=AF.Identity,
                                 bias=b2c_sb[:])
            nc.sync.dma_start(out_d[:], outS[:])
            psC_cm.__exit__(None, None, None)
    return nc


LAST_EXEC_NS = None


def kernel(**inputs):
    global LAST_EXEC_NS
    import time
    dbg = os.environ.get("KBENCH") == "1"
    t0 = time.time()
    T, rep, per_core = _host_prep(inputs)
    t1 = time.time()
    nc = _build_program(T)
    in_maps = [dict(rep, **per_core[c]) for c in range(NCORES)]
    nc.finalize()
    t2 = time.time()
    if dbg:
        print(f"[kbench] host_prep={t1-t0:.2f}s build+finalize={t2-t1:.2f}s", flush=True)
        import jax
        jax.devices()
        t3 = time.time()
        print(f"[kbench] jax backend init={t3-t2:.2f}s", flush=True)
    trace = os.environ.get("BASS_TRACE") == "1"
    res = run_bass_kernel_spmd(nc, in_maps, list(range(NCORES)), trace=trace)
    t4 = time.time()
    if dbg:
        print(f"[kbench] run_spmd={t4-t2:.2f}s", flush=True)
    LAST_EXEC_NS = getattr(res, "exec_time_ns", None)
    return np.asarray(res.results[0]["out"], dtype=np.float32)

